# revision 36
# baseline (speedup 1.0000x reference)
"""Trainium2 Bass kernel for nn_Attention (dense transformer block attention).

Strategy: data-parallel over the batch axis — 128 windows / 8 cores = 16 per
core; weights + bias table replicated. No collectives.

Per-core pipeline (all matmuls bf16, f32 accumulate):
  - x arrives pre-transposed per batch: xT [768, 256] (features on partitions)
  - QK-proj (layout B):  qkT[cols, tok] = W_chunk^T @ xT_chunk   -> qT/kT [d, tok]
  - V-proj  (layout A):  v[tok, cols]   = xT_chunk^T @ Wv_chunk  -> V [m, d]
  - scores: psum <- identity@bias (bias pre-gathered), += qT_h^T @ kT_h (K=64)
  - exp on scalar engine with fused accum_out -> denominators
  - normalize on vector engine, bf16 attn -> DMA-transpose -> AT [m, n]
  - AV: outT[d, n] = V_h^T.T @ AT_h  (col-packed head pairs) -> attnoutT
  - out-proj: out[tok, e] = attnoutT_chunk^T @ Wout_chunk (+ones-row bias)
  - rel-pos bias: gpsimd indirect_copy over a PE-transposed per-head table
    (heads on partitions mod 16), 8 chunks of 1024 idxs, bf16 DRAM bounce,
    stride-decomposed reload into [128, 12, 512] scores layout.
"""

import numpy as np

N_CORES = 8
B, N, E = 128, 256, 768
H, D = 12, 64
BPC = B // N_CORES  # batches per core
NT = N // 128       # token tiles per batch (2)
KF = E // 128       # feature chunks (6)
SCALE = 1.0 / 8.0   # d ** -0.5

_CACHE = {}


def _make_idsw(ids: np.ndarray) -> np.ndarray:
    """Wrapped gather indices for indirect_copy (pure data movement).

    GPSIMD group c (partitions 16c..16c+15) gathers position chunk c
    ([c*8192, (c+1)*8192)); call k covers local positions [k*1024, (k+1)*1024).
    Index i of a call sits at [16c + i%16, k*64 + i//16].
    """
    ids8 = ids.reshape(8, 8, 1024)               # [c, k, i]
    idsw = np.zeros((128, 4096), np.int32)
    for c in range(8):
        for k in range(8):
            idsw[c * 16:(c + 1) * 16, k * 64:(k + 1) * 64] = \
                ids8[c, k].reshape(64, 16).T
    return idsw


def _split_multi_waits(nc, mybir):
    """Hoist extra semaphore waits onto standalone EventSemaphore instructions.

    The TPB ISA encodes exactly one wait per instruction (EventSemaphore: two);
    this walrus build rejects instructions with more. Tile emits multi-wait
    instructions, so split them: extra waits become wait-only instructions on
    the same engine, immediately before, preserving semantics.
    """
    for f in nc.m.functions:
        for blk in f.blocks:
            insts = list(blk.instructions)
            new_insts = []
            for ins in insts:
                si = ins.sync_info
                cap = 2 if isinstance(ins, mybir.InstEventSemaphore) else 1
                if si is not None and len(si.on_wait) > cap:
                    waits = list(si.on_wait)
                    extra, keep = waits[:-cap], waits[-cap:]
                    for i in range(0, len(extra), 2):
                        ev = mybir.InstEventSemaphore(
                            name=f"WSPLIT-{nc.next_id()}", ins=[], outs=[]
                        )
                        ev.engine = ins.engine
                        ev.sync_info = mybir.SyncInfo(
                            on_wait=extra[i:i + 2], on_update=[]
                        )
                        new_insts.append(ev)
                    ins.sync_info = mybir.SyncInfo(
                        on_wait=keep, on_update=list(si.on_update)
                    )
                new_insts.append(ins)
            blk.instructions = new_insts


def _build(n_batch: int, sim: bool = False):
    import concourse.bass as bass
    import concourse.mybir as mybir
    from concourse.tile import TileContext
    from concourse.masks import make_identity

    f32 = mybir.dt.float32
    bf16 = mybir.dt.bfloat16
    i32 = mybir.dt.int32
    u16 = mybir.dt.uint16
    AF = mybir.ActivationFunctionType

    nc = bass.Bass(detect_race_conditions=False)

    xT_ext = nc.declare_dram_parameter("xT", [n_batch, E, N], f32, isOutput=False)
    wqkv_ext = nc.declare_dram_parameter("wqkv", [E, 3 * E], f32, isOutput=False)
    bqkv_ext = nc.declare_dram_parameter("bqkv", [3 * E], f32, isOutput=False)
    wout_ext = nc.declare_dram_parameter("wout", [E, E], f32, isOutput=False)
    bout_ext = nc.declare_dram_parameter("bout", [E], f32, isOutput=False)
    table_ext = nc.declare_dram_parameter("table", [961, H], f32, isOutput=False)
    idsw_ext = nc.declare_dram_parameter("idsw", [128, 4096], i32, isOutput=False)
    out_ext = nc.declare_dram_parameter("out", [n_batch * N, E], f32, isOutput=True)

    with TileContext(nc) as tc:
        with (
            tc.tile_pool(name="const", bufs=1) as constp,
            tc.tile_pool(name="wts", bufs=1) as wts,
            tc.tile_pool(name="scratch", bufs=1) as scratch,
            tc.tile_pool(name="dram", bufs=1, space="DRAM") as dramp,
            tc.tile_pool(name="xin", bufs=2) as xin,
            tc.tile_pool(name="qk", bufs=3) as qkp,
            tc.tile_pool(name="vp", bufs=2) as vpool,
            tc.tile_pool(name="expp", bufs=4) as expp,
            tc.tile_pool(name="attn", bufs=2) as attnp,
            tc.tile_pool(name="atp", bufs=2) as atp,
            tc.tile_pool(name="aop", bufs=2) as aop,
            tc.tile_pool(name="outp", bufs=1) as outp,
            tc.tile_pool(name="ps", bufs=8, space="PSUM") as ps,
        ):
            # ---------------- constants / weights ----------------
            ident = constp.tile([128, 128], bf16)
            make_identity(nc, ident[:])

            ones_row = constp.tile([1, 256], bf16)
            nc.vector.memset(ones_row[:], 1.0)

            # ---------------- rel-pos bias gather ----------------
            ids32 = scratch.tile([128, 4096], i32, tag="scr")
            nc.sync.dma_start(out=ids32[:], in_=idsw_ext[:, :])
            idx16 = constp.tile([128, 4096], u16)
            ids16v = ids32[:].bitcast(u16).rearrange("p (c two) -> p c two", two=2)
            nc.vector.tensor_copy(idx16[:], ids16v[:, :, 0])

            # tableT[16c+e, j] = table[j, e] (e<12; heads on partitions mod
            # 16, replicated per gpsimd group). Built via PE transposes.
            tableT = constp.tile([128, 961], f32)
            nc.vector.memset(tableT[:], 0.0)
            tab_sb = scratch.tile([128, 8, H], f32, tag="scr")
            nc.vector.memset(tab_sb[:], 0.0)
            nc.sync.dma_start(
                out=tab_sb[:, 0:7, :],
                in_=table_ext[0:896].rearrange("(k p) e -> p k e", p=128),
            )
            nc.sync.dma_start(out=tab_sb[0:65, 7, :], in_=table_ext[896:961, :])
            idf32 = constp.tile([128, 128], f32)
            make_identity(nc, idf32[:])
            for k in range(8):
                pst = ps.tile([128, 128], f32, tag="ps")
                nc.tensor.transpose(pst[0:H, :], tab_sb[:, k, :], idf32[:])
                w = 961 - k * 128 if k == 7 else 128
                nc.vector.tensor_copy(
                    tableT[0:H, k * 128:k * 128 + w], pst[0:H, 0:w]
                )
            for c in range(1, 8):
                nc.sync.dma_start(
                    out=tableT[c * 16:c * 16 + 16, :], in_=tableT[0:16, :]
                )

            # 8 chunked gathers (<=1024 dst elems each) -> bf16 -> DRAM bounce
            # bias_dram flat addr = ((16c + e) * 8192 + j) where group c holds
            # positions [c*8192, (c+1)*8192), e = head, j = local position
            bias_dram = dramp.tile([128, 8192], bf16)
            for k in range(8):
                gth = scratch.tile([128, 1024], f32, tag="gth")
                nc.gpsimd.indirect_copy(
                    gth[:], tableT[:], idx16[:, k * 64:(k + 1) * 64], True
                )
                gth16 = scratch.tile([128, 1024], bf16, tag="gth16")
                nc.vector.tensor_copy(gth16[:], gth[:])
                nc.sync.dma_start(
                    out=bias_dram[:, k * 1024:(k + 1) * 1024], in_=gth16[:]
                )

            # reload into scores layout: bias_sb[p, h, t*256+m] =
            # table[ids[(t*128+p)*256+m], h]. Flat DRAM index decomposes as
            # [t, r, hh, q, m] with p = 32r + q -> 4x2 partition-striped DMAs
            bias_sb = constp.tile([128, H, 512], bf16)
            bd5 = bias_dram[:].rearrange("a b -> (a b)").rearrange(
                "(t r hh q m) -> t r q hh m", t=2, r=4, hh=16, q=32, m=256
            )
            for r in range(4):
                for t in range(NT):
                    nc.sync.dma_start(
                        out=bias_sb[r * 32:(r + 1) * 32, :, t * 256:(t + 1) * 256],
                        in_=bd5[t, r, :, 0:H, :],
                    )


            # wqkv -> bf16 [128, KF, 2304]
            wqkv_sb = wts.tile([128, KF, 3 * E], bf16)
            for f in range(KF):
                st = scratch.tile([128, 3 * E], f32, tag="scr")
                nc.sync.dma_start(out=st[:], in_=wqkv_ext[f * 128:(f + 1) * 128, :])
                nc.vector.tensor_copy(wqkv_sb[:, f, :], st[:])

            # wout -> bf16 [128, KF, 768]
            wout_sb = wts.tile([128, KF, E], bf16)
            for f in range(KF):
                st = scratch.tile([128, E], f32, tag="scr")
                nc.sync.dma_start(out=st[:], in_=wout_ext[f * 128:(f + 1) * 128, :])
                nc.vector.tensor_copy(wout_sb[:, f, :], st[:])

            # b_qkv / b_out as bf16 rows on partition 0; v/out biases enter
            # their psums via K=1 matmuls (ones-row outer products)
            bqkv_raw = constp.tile([1, 3 * E], f32)
            nc.sync.dma_start(out=bqkv_raw[:], in_=bqkv_ext[:][None, :])
            bqkv_row = constp.tile([1, 3 * E], bf16)
            nc.vector.tensor_copy(bqkv_row[:], bqkv_raw[:])

            # qk bias as a per-partition vector bqk[p, ch] = bqkv[ch*128+p]
            # (qkT psum partitions = qkv cols) via one PE transpose; the
            # q-half is pre-scaled so the evac's out = psum*s + bias*s
            bq12 = constp.tile([12, 128], f32)
            nc.sync.dma_start(
                out=bq12[:], in_=bqkv_ext[0:2 * E].rearrange("(c p) -> c p", p=128)
            )
            bqk_ps = ps.tile([128, 12], f32, tag="ps")
            nc.tensor.transpose(bqk_ps[:], bq12[:], idf32[0:12, 0:12])
            bqk_sb = constp.tile([128, 12], f32)
            nc.scalar.mul(bqk_sb[:, 0:KF], bqk_ps[:, 0:KF], SCALE)
            nc.scalar.copy(bqk_sb[:, KF:2 * KF], bqk_ps[:, KF:2 * KF])

            bo_raw = constp.tile([1, E], f32)
            nc.sync.dma_start(out=bo_raw[:], in_=bout_ext[:][None, :])
            bo_row = constp.tile([1, E], bf16)
            nc.vector.tensor_copy(bo_row[:], bo_raw[:])

            # ---------------- main loop over batches ----------------
            for b in range(n_batch):
                # xT load + cast
                xf = xin.tile([128, KF, N], f32, tag="xf")
                nc.sync.dma_start(
                    out=xf[:],
                    in_=xT_ext[b].rearrange("(f p) n -> p f n", p=128),
                )
                xT = xin.tile([128, KF, N], bf16, tag="xT")
                nc.vector.tensor_copy(xT[:], xf[:])

                # QK projection: qkT [128, 12, 256]
                qkT = qkp.tile([128, 2 * KF, N], bf16)
                for ch in range(2 * KF):
                    pq = ps.tile([128, N], f32, tag="ps")
                    for f in range(KF):
                        nc.tensor.matmul(
                            pq[:],
                            lhsT=wqkv_sb[:, f, ch * 128:(ch + 1) * 128],
                            rhs=xT[:, f, :],
                            start=(f == 0),
                            stop=(f == KF - 1),
                        )
                    nc.scalar.activation(
                        qkT[:, ch, :], pq[:], AF.Identity,
                        bias=bqk_sb[:, ch:ch + 1],
                        scale=SCALE if ch < KF else 1.0,
                    )

                # V projection: v_sb [128, NT, 768] (tokens on partitions)
                v_sb = vpool.tile([128, NT, E], bf16)
                for t in range(NT):
                    for n0, nw in ((0, 512), (512, 256)):
                        pv = ps.tile([128, nw], f32, tag="ps")
                        for f in range(KF):
                            nc.tensor.matmul(
                                pv[:],
                                lhsT=xT[:, f, t * 128:(t + 1) * 128],
                                rhs=wqkv_sb[:, f, 2 * E + n0:2 * E + n0 + nw],
                                start=(f == 0),
                                stop=False,
                            )
                        nc.tensor.matmul(
                            pv[:],
                            lhsT=ones_row[:, 0:128],
                            rhs=bqkv_row[:, 2 * E + n0:2 * E + n0 + nw],
                            start=False,
                            stop=True,
                        )
                        nc.vector.tensor_copy(v_sb[:, t, n0:n0 + nw], pv[:])

                # scores + exp + normalize per (t, h)
                attn = attnp.tile([128, NT * H, N], bf16)
                denom = expp.tile([128, NT * H], f32, tag="den")
                recip = expp.tile([128, NT * H], f32, tag="rec")
                for t in range(NT):
                    for h in range(H):
                        po = (h % 2) * 64
                        hc = h // 2
                        pss = ps.tile([128, N], f32, tag="ps")
                        nc.tensor.matmul(
                            pss[:],
                            lhsT=qkT[po:po + 64, hc, t * 128:(t + 1) * 128],
                            rhs=qkT[po:po + 64, KF + hc, :],
                            start=True,
                            stop=True,
                        )
                        nc.vector.tensor_add(
                            pss[:], pss[:], bias_sb[:, h, t * 256:(t + 1) * 256]
                        )
                        nc.scalar.activation(
                            attn[:, t * H + h, :], pss[:], AF.Exp,
                            accum_out=denom[:, t * H + h:t * H + h + 1],
                        )
                    nc.vector.reciprocal(
                        recip[:, t * H:(t + 1) * H], denom[:, t * H:(t + 1) * H]
                    )
                    for h in range(H):
                        nc.vector.tensor_scalar_mul(
                            attn[:, t * H + h, :],
                            attn[:, t * H + h, :],
                            recip[:, t * H + h:t * H + h + 1],
                        )

                # transpose attn -> AT [128, NT, 2H, 128]
                AT = atp.tile([128, NT, 2 * H, 128], bf16)
                for t in range(NT):
                    nc.sync.dma_start(
                        out=AT[:, t],
                        in_=attn[:, t * H:(t + 1) * H, :].rearrange("p h n -> p (h n)"),
                        transpose=True,
                    )

                # AV: attnoutT [128, KF, 256] (feature rows = head-pairs)
                aoT = aop.tile([128, KF, N], bf16)
                for hp in range(KF):
                    pav = ps.tile([128, N], f32, tag="ps")
                    for po, h in ((0, 2 * hp), (64, 2 * hp + 1)):
                        for u in range(NT):
                            nc.tensor.matmul(
                                pav[po:po + 64, :],
                                lhsT=v_sb[:, u, h * 64:(h + 1) * 64],
                                rhs=AT[:, :, h * 2 + u, :],
                                start=(u == 0),
                                stop=(u == NT - 1),
                                tile_position=(0, po),
                            )
                    nc.vector.tensor_copy(aoT[:, hp, :], pav[:])

                # out projection
                osb = outp.tile([128, NT, E], f32)
                for t in range(NT):
                    for n0, nw in ((0, 512), (512, 256)):
                        pso = ps.tile([128, nw], f32, tag="ps")
                        for f in range(KF):
                            nc.tensor.matmul(
                                pso[:],
                                lhsT=aoT[:, f, t * 128:(t + 1) * 128],
                                rhs=wout_sb[:, f, n0:n0 + nw],
                                start=(f == 0),
                                stop=False,
                            )
                        nc.tensor.matmul(
                            pso[:],
                            lhsT=ones_row[:, 0:128],
                            rhs=bo_row[:, n0:n0 + nw],
                            start=False,
                            stop=True,
                        )
                        nc.vector.tensor_copy(osb[:, t, n0:n0 + nw], pso[:])

                nc.sync.dma_start(
                    out=out_ext[b * N:(b + 1) * N, :].rearrange(
                        "(t p) e -> p t e", p=128
                    ),
                    in_=osb[:],
                )

    _split_multi_waits(nc, mybir)
    return nc


def kernel(x, W_qkv, b_qkv, W_out, b_out, rel_bias_table, rel_pos_ids):
    from concourse.bass_utils import run_bass_kernel_spmd

    x = np.ascontiguousarray(np.asarray(x, dtype=np.float32))
    W_qkv = np.ascontiguousarray(np.asarray(W_qkv, dtype=np.float32))
    b_qkv = np.ascontiguousarray(np.asarray(b_qkv, dtype=np.float32))
    W_out = np.ascontiguousarray(np.asarray(W_out, dtype=np.float32))
    b_out = np.ascontiguousarray(np.asarray(b_out, dtype=np.float32))
    table = np.ascontiguousarray(np.asarray(rel_bias_table, dtype=np.float32))
    ids = np.asarray(rel_pos_ids, dtype=np.int32).reshape(-1)

    idsw = _make_idsw(ids)

    if "nc" not in _CACHE:
        _CACHE["nc"] = _build(BPC)
    nc = _CACHE["nc"]

    in_maps = []
    for c in range(N_CORES):
        xs = x[c * BPC:(c + 1) * BPC]                      # [BPC, 256, 768]
        xT = np.ascontiguousarray(xs.transpose(0, 2, 1))   # [BPC, 768, 256]
        in_maps.append({
            "xT": xT,
            "wqkv": W_qkv,
            "bqkv": b_qkv,
            "wout": W_out,
            "bout": b_out,
            "table": table,
            "idsw": idsw,
        })

    _CACHE["in_maps"] = in_maps
    res = run_bass_kernel_spmd(nc, in_maps, core_ids=list(range(N_CORES)))
    out = np.stack([r["out"] for r in res.results], axis=0)  # [8, BPC*256, 768]
    return out.reshape(B, N, E).astype(np.float32)


if __name__ == "__main__":
    rng = np.random.default_rng(0)
    ids = rng.integers(0, 961, 65536).astype(np.int32)
    table = rng.standard_normal((961, H)).astype(np.float32)
    idsw = _make_idsw(ids)
    # emulate indirect_copy + DRAM bounce + strided reload
    bias_dram = np.zeros((128, 8192), np.float32)
    for c in range(8):
        for k in range(8):
            blk = idsw[c * 16:(c + 1) * 16, k * 64:(k + 1) * 64]
            unwrapped = blk.T.reshape(-1)  # [1024]
            for e in range(H):
                bias_dram[c * 16 + e, k * 1024:(k + 1) * 1024] = \
                    table[unwrapped, e]
    flat = bias_dram.reshape(-1)
    bd5 = flat.reshape(2, 4, 16, 32, 256).transpose(0, 1, 3, 2, 4)
    bias_sb = np.zeros((128, H, 512), np.float32)
    for r in range(4):
        for t in range(2):
            bias_sb[r * 32:(r + 1) * 32, :, t * 256:(t + 1) * 256] = \
                bd5[t, r, :, 0:H, :]
    # check vs reference layout: bias_sb[p, h, t*256+m] == bias[h, t*128+p, m]
    ref = table[ids].reshape(256, 256, H).transpose(2, 0, 1)
    err = 0.0
    for t in range(2):
        for p in range(128):
            err = max(err, np.abs(
                bias_sb[p, :, t * 256:(t + 1) * 256] - ref[:, t * 128 + p, :]
            ).max())
    print("bias gather layout check:", err)


# revision 38
# speedup vs baseline: 1.0109x; 1.0109x over previous
"""Trainium2 Bass kernel for nn_Attention (dense transformer block attention).

Strategy: data-parallel over the batch axis — 128 windows / 8 cores = 16 per
core; weights + bias table replicated. No collectives.

Per-core pipeline (all matmuls bf16, f32 accumulate):
  - x arrives pre-transposed per batch: xT [768, 256] (features on partitions)
  - QK-proj (layout B):  qkT[cols, tok] = W_chunk^T @ xT_chunk   -> qT/kT [d, tok]
  - V-proj  (layout A):  v[tok, cols]   = xT_chunk^T @ Wv_chunk  -> V [m, d]
  - scores: psum <- identity@bias (bias pre-gathered), += qT_h^T @ kT_h (K=64)
  - exp on scalar engine with fused accum_out -> denominators
  - normalize on vector engine, bf16 attn -> DMA-transpose -> AT [m, n]
  - AV: outT[d, n] = V_h^T.T @ AT_h  (col-packed head pairs) -> attnoutT
  - out-proj: out[tok, e] = attnoutT_chunk^T @ Wout_chunk (+ones-row bias)
  - rel-pos bias: gpsimd indirect_copy over a PE-transposed per-head table
    (heads on partitions mod 16), 8 chunks of 1024 idxs, bf16 DRAM bounce,
    stride-decomposed reload into [128, 12, 512] scores layout.
"""

import numpy as np

N_CORES = 8
B, N, E = 128, 256, 768
H, D = 12, 64
BPC = B // N_CORES  # batches per core
NT = N // 128       # token tiles per batch (2)
KF = E // 128       # feature chunks (6)
SCALE = 1.0 / 8.0   # d ** -0.5

_CACHE = {}


def _make_idsw(ids: np.ndarray) -> np.ndarray:
    """Wrapped gather indices for indirect_copy (pure data movement).

    GPSIMD group c (partitions 16c..16c+15) gathers position chunk c
    ([c*8192, (c+1)*8192)); call k covers local positions [k*1024, (k+1)*1024).
    Index i of a call sits at [16c + i%16, k*64 + i//16].
    """
    ids8 = ids.reshape(8, 8, 1024)               # [c, k, i]
    idsw = np.zeros((128, 4096), np.int32)
    for c in range(8):
        for k in range(8):
            idsw[c * 16:(c + 1) * 16, k * 64:(k + 1) * 64] = \
                ids8[c, k].reshape(64, 16).T
    return idsw


def _split_multi_waits(nc, mybir):
    """Hoist extra semaphore waits onto standalone EventSemaphore instructions.

    The TPB ISA encodes exactly one wait per instruction (EventSemaphore: two);
    this walrus build rejects instructions with more. Tile emits multi-wait
    instructions, so split them: extra waits become wait-only instructions on
    the same engine, immediately before, preserving semantics.
    """
    for f in nc.m.functions:
        for blk in f.blocks:
            insts = list(blk.instructions)
            new_insts = []
            for ins in insts:
                si = ins.sync_info
                cap = 2 if isinstance(ins, mybir.InstEventSemaphore) else 1
                if si is not None and len(si.on_wait) > cap:
                    waits = list(si.on_wait)
                    extra, keep = waits[:-cap], waits[-cap:]
                    for i in range(0, len(extra), 2):
                        ev = mybir.InstEventSemaphore(
                            name=f"WSPLIT-{nc.next_id()}", ins=[], outs=[]
                        )
                        ev.engine = ins.engine
                        ev.sync_info = mybir.SyncInfo(
                            on_wait=extra[i:i + 2], on_update=[]
                        )
                        new_insts.append(ev)
                    ins.sync_info = mybir.SyncInfo(
                        on_wait=keep, on_update=list(si.on_update)
                    )
                new_insts.append(ins)
            blk.instructions = new_insts


def _build(n_batch: int, sim: bool = False):
    import concourse.bass as bass
    import concourse.mybir as mybir
    from concourse.tile import TileContext
    from concourse.masks import make_identity

    f32 = mybir.dt.float32
    bf16 = mybir.dt.bfloat16
    i32 = mybir.dt.int32
    u16 = mybir.dt.uint16
    AF = mybir.ActivationFunctionType

    nc = bass.Bass(detect_race_conditions=False)

    xT_ext = nc.declare_dram_parameter("xT", [n_batch, E, N], f32, isOutput=False)
    wqkv_ext = nc.declare_dram_parameter("wqkv", [E, 3 * E], f32, isOutput=False)
    bqkv_ext = nc.declare_dram_parameter("bqkv", [3 * E], f32, isOutput=False)
    wout_ext = nc.declare_dram_parameter("wout", [E, E], f32, isOutput=False)
    bout_ext = nc.declare_dram_parameter("bout", [E], f32, isOutput=False)
    table_ext = nc.declare_dram_parameter("table", [961, H], f32, isOutput=False)
    idsw_ext = nc.declare_dram_parameter("idsw", [128, 4096], i32, isOutput=False)
    out_ext = nc.declare_dram_parameter("out", [n_batch * N, E], f32, isOutput=True)

    with TileContext(nc) as tc:
        with (
            tc.tile_pool(name="const", bufs=1) as constp,
            tc.tile_pool(name="wts", bufs=1) as wts,
            tc.tile_pool(name="scratch", bufs=1) as scratch,
            tc.tile_pool(name="dram", bufs=1, space="DRAM") as dramp,
            tc.tile_pool(name="xin", bufs=2) as xin,
            tc.tile_pool(name="qk", bufs=3) as qkp,
            tc.tile_pool(name="vp", bufs=2) as vpool,
            tc.tile_pool(name="expp", bufs=4) as expp,
            tc.tile_pool(name="attn", bufs=2) as attnp,
            tc.tile_pool(name="atp", bufs=2) as atp,
            tc.tile_pool(name="aop", bufs=2) as aop,
            tc.tile_pool(name="outp", bufs=1) as outp,
            tc.tile_pool(name="ps", bufs=8, space="PSUM") as ps,
        ):
            # ---------------- constants / weights ----------------
            ident = constp.tile([128, 128], bf16)
            make_identity(nc, ident[:])

            # PE warmup: ~8us of dummy matmuls trips the HAM activity window
            # so the real work starts at full clock instead of K=4/8
            pwarm = ps.tile([128, 128], f32, tag="ps")
            for wi in range(40):
                nc.tensor.matmul(
                    pwarm[:], lhsT=ident[:], rhs=ident[:],
                    start=(wi == 0), stop=(wi == 39),
                )

            ones_row = constp.tile([1, 256], bf16)
            nc.vector.memset(ones_row[:], 1.0)

            # ---------------- rel-pos bias gather ----------------
            ids32 = scratch.tile([128, 4096], i32, tag="scr")
            nc.sync.dma_start(out=ids32[:], in_=idsw_ext[:, :])
            idx16 = constp.tile([128, 4096], u16)
            ids16v = ids32[:].bitcast(u16).rearrange("p (c two) -> p c two", two=2)
            nc.vector.tensor_copy(idx16[:], ids16v[:, :, 0])

            # tableT[16c+e, j] = table[j, e] (e<12; heads on partitions mod
            # 16, replicated per gpsimd group). Built via PE transposes.
            tableT = constp.tile([128, 961], f32)
            nc.vector.memset(tableT[:], 0.0)
            tab_sb = scratch.tile([128, 8, H], f32, tag="scr")
            nc.vector.memset(tab_sb[:], 0.0)
            nc.sync.dma_start(
                out=tab_sb[:, 0:7, :],
                in_=table_ext[0:896].rearrange("(k p) e -> p k e", p=128),
            )
            nc.sync.dma_start(out=tab_sb[0:65, 7, :], in_=table_ext[896:961, :])
            idf32 = constp.tile([128, 128], f32)
            make_identity(nc, idf32[:])
            for k in range(8):
                pst = ps.tile([128, 128], f32, tag="ps")
                nc.tensor.transpose(pst[0:H, :], tab_sb[:, k, :], idf32[:])
                w = 961 - k * 128 if k == 7 else 128
                nc.vector.tensor_copy(
                    tableT[0:H, k * 128:k * 128 + w], pst[0:H, 0:w]
                )
            for c in range(1, 8):
                nc.sync.dma_start(
                    out=tableT[c * 16:c * 16 + 16, :], in_=tableT[0:16, :]
                )

            # 8 chunked gathers (<=1024 dst elems each) -> bf16 -> DRAM bounce
            # bias_dram flat addr = ((16c + e) * 8192 + j) where group c holds
            # positions [c*8192, (c+1)*8192), e = head, j = local position
            bias_dram = dramp.tile([128, 8192], bf16)
            for k in range(8):
                gth = scratch.tile([128, 1024], f32, tag="gth")
                nc.gpsimd.indirect_copy(
                    gth[:], tableT[:], idx16[:, k * 64:(k + 1) * 64], True
                )
                gth16 = scratch.tile([128, 1024], bf16, tag="gth16")
                nc.vector.tensor_copy(gth16[:], gth[:])
                nc.sync.dma_start(
                    out=bias_dram[:, k * 1024:(k + 1) * 1024], in_=gth16[:]
                )

            # reload into scores layout: bias_sb[p, h, t*256+m] =
            # table[ids[(t*128+p)*256+m], h]. Flat DRAM index decomposes as
            # [t, r, hh, q, m] with p = 32r + q -> 4x2 partition-striped DMAs
            bias_sb = constp.tile([128, H, 512], bf16)
            bd5 = bias_dram[:].rearrange("a b -> (a b)").rearrange(
                "(t r hh q m) -> t r q hh m", t=2, r=4, hh=16, q=32, m=256
            )
            for r in range(4):
                for t in range(NT):
                    nc.sync.dma_start(
                        out=bias_sb[r * 32:(r + 1) * 32, :, t * 256:(t + 1) * 256],
                        in_=bd5[t, r, :, 0:H, :],
                    )


            # wqkv -> bf16 [128, KF, 2304]
            wqkv_sb = wts.tile([128, KF, 3 * E], bf16)
            for f in range(KF):
                st = scratch.tile([128, 3 * E], f32, tag="scr")
                nc.sync.dma_start(out=st[:], in_=wqkv_ext[f * 128:(f + 1) * 128, :])
                nc.vector.tensor_copy(wqkv_sb[:, f, :], st[:])

            # wout -> bf16 [128, KF, 768]
            wout_sb = wts.tile([128, KF, E], bf16)
            for f in range(KF):
                st = scratch.tile([128, E], f32, tag="scr")
                nc.sync.dma_start(out=st[:], in_=wout_ext[f * 128:(f + 1) * 128, :])
                nc.vector.tensor_copy(wout_sb[:, f, :], st[:])

            # b_qkv / b_out as bf16 rows on partition 0; v/out biases enter
            # their psums via K=1 matmuls (ones-row outer products)
            bqkv_raw = constp.tile([1, 3 * E], f32)
            nc.sync.dma_start(out=bqkv_raw[:], in_=bqkv_ext[:][None, :])
            bqkv_row = constp.tile([1, 3 * E], bf16)
            nc.vector.tensor_copy(bqkv_row[:], bqkv_raw[:])

            # qk bias as a per-partition vector bqk[p, ch] = bqkv[ch*128+p]
            # (qkT psum partitions = qkv cols) via one PE transpose; the
            # q-half is pre-scaled so the evac's out = psum*s + bias*s
            bq12 = constp.tile([12, 128], f32)
            nc.sync.dma_start(
                out=bq12[:], in_=bqkv_ext[0:2 * E].rearrange("(c p) -> c p", p=128)
            )
            bqk_ps = ps.tile([128, 12], f32, tag="ps")
            nc.tensor.transpose(bqk_ps[:], bq12[:], idf32[0:12, 0:12])
            bqk_sb = constp.tile([128, 12], f32)
            nc.scalar.mul(bqk_sb[:, 0:KF], bqk_ps[:, 0:KF], SCALE)
            nc.scalar.copy(bqk_sb[:, KF:2 * KF], bqk_ps[:, KF:2 * KF])

            bo_raw = constp.tile([1, E], f32)
            nc.sync.dma_start(out=bo_raw[:], in_=bout_ext[:][None, :])
            bo_row = constp.tile([1, E], bf16)
            nc.vector.tensor_copy(bo_row[:], bo_raw[:])

            # ---------------- main loop over batches ----------------
            for b in range(n_batch):
                # xT load + cast
                xf = xin.tile([128, KF, N], f32, tag="xf")
                nc.sync.dma_start(
                    out=xf[:],
                    in_=xT_ext[b].rearrange("(f p) n -> p f n", p=128),
                )
                xT = xin.tile([128, KF, N], bf16, tag="xT")
                nc.vector.tensor_copy(xT[:], xf[:])

                # QK projection: qkT [128, 12, 256]
                qkT = qkp.tile([128, 2 * KF, N], bf16)
                for ch in range(2 * KF):
                    pq = ps.tile([128, N], f32, tag="ps")
                    for f in range(KF):
                        nc.tensor.matmul(
                            pq[:],
                            lhsT=wqkv_sb[:, f, ch * 128:(ch + 1) * 128],
                            rhs=xT[:, f, :],
                            start=(f == 0),
                            stop=(f == KF - 1),
                        )
                    nc.scalar.activation(
                        qkT[:, ch, :], pq[:], AF.Identity,
                        bias=bqk_sb[:, ch:ch + 1],
                        scale=SCALE if ch < KF else 1.0,
                    )

                # V projection: v_sb [128, NT, 768] (tokens on partitions)
                v_sb = vpool.tile([128, NT, E], bf16)
                for t in range(NT):
                    for n0, nw in ((0, 512), (512, 256)):
                        pv = ps.tile([128, nw], f32, tag="ps")
                        for f in range(KF):
                            nc.tensor.matmul(
                                pv[:],
                                lhsT=xT[:, f, t * 128:(t + 1) * 128],
                                rhs=wqkv_sb[:, f, 2 * E + n0:2 * E + n0 + nw],
                                start=(f == 0),
                                stop=False,
                            )
                        nc.tensor.matmul(
                            pv[:],
                            lhsT=ones_row[:, 0:128],
                            rhs=bqkv_row[:, 2 * E + n0:2 * E + n0 + nw],
                            start=False,
                            stop=True,
                        )
                        nc.vector.tensor_copy(v_sb[:, t, n0:n0 + nw], pv[:])

                # scores + exp + normalize per (t, h)
                attn = attnp.tile([128, NT * H, N], bf16)
                denom = expp.tile([128, NT * H], f32, tag="den")
                recip = expp.tile([128, NT * H], f32, tag="rec")
                for t in range(NT):
                    for h in range(H):
                        po = (h % 2) * 64
                        hc = h // 2
                        pss = ps.tile([128, N], f32, tag="ps")
                        nc.tensor.matmul(
                            pss[:],
                            lhsT=qkT[po:po + 64, hc, t * 128:(t + 1) * 128],
                            rhs=qkT[po:po + 64, KF + hc, :],
                            start=True,
                            stop=False,
                        )
                        nc.tensor.matmul(
                            pss[:],
                            lhsT=ident[:],
                            rhs=bias_sb[:, h, t * 256:(t + 1) * 256],
                            start=False,
                            stop=True,
                        )
                        nc.scalar.activation(
                            attn[:, t * H + h, :], pss[:], AF.Exp,
                            accum_out=denom[:, t * H + h:t * H + h + 1],
                        )
                    nc.vector.reciprocal(
                        recip[:, t * H:(t + 1) * H], denom[:, t * H:(t + 1) * H]
                    )
                    for h in range(H):
                        nc.vector.tensor_scalar_mul(
                            attn[:, t * H + h, :],
                            attn[:, t * H + h, :],
                            recip[:, t * H + h:t * H + h + 1],
                        )

                # transpose attn -> AT [128, NT, 2H, 128]
                AT = atp.tile([128, NT, 2 * H, 128], bf16)
                for t in range(NT):
                    nc.sync.dma_start(
                        out=AT[:, t],
                        in_=attn[:, t * H:(t + 1) * H, :].rearrange("p h n -> p (h n)"),
                        transpose=True,
                    )

                # AV: attnoutT [128, KF, 256] (feature rows = head-pairs)
                aoT = aop.tile([128, KF, N], bf16)
                for hp in range(KF):
                    pav = ps.tile([128, N], f32, tag="ps")
                    for po, h in ((0, 2 * hp), (64, 2 * hp + 1)):
                        for u in range(NT):
                            nc.tensor.matmul(
                                pav[po:po + 64, :],
                                lhsT=v_sb[:, u, h * 64:(h + 1) * 64],
                                rhs=AT[:, :, h * 2 + u, :],
                                start=(u == 0),
                                stop=(u == NT - 1),
                                tile_position=(0, po),
                            )
                    nc.vector.tensor_copy(aoT[:, hp, :], pav[:])

                # out projection
                osb = outp.tile([128, NT, E], f32)
                for t in range(NT):
                    for n0, nw in ((0, 512), (512, 256)):
                        pso = ps.tile([128, nw], f32, tag="ps")
                        for f in range(KF):
                            nc.tensor.matmul(
                                pso[:],
                                lhsT=aoT[:, f, t * 128:(t + 1) * 128],
                                rhs=wout_sb[:, f, n0:n0 + nw],
                                start=(f == 0),
                                stop=False,
                            )
                        nc.tensor.matmul(
                            pso[:],
                            lhsT=ones_row[:, 0:128],
                            rhs=bo_row[:, n0:n0 + nw],
                            start=False,
                            stop=True,
                        )
                        nc.vector.tensor_copy(osb[:, t, n0:n0 + nw], pso[:])

                nc.sync.dma_start(
                    out=out_ext[b * N:(b + 1) * N, :].rearrange(
                        "(t p) e -> p t e", p=128
                    ),
                    in_=osb[:],
                )

    _split_multi_waits(nc, mybir)
    return nc


def kernel(x, W_qkv, b_qkv, W_out, b_out, rel_bias_table, rel_pos_ids):
    from concourse.bass_utils import run_bass_kernel_spmd

    x = np.ascontiguousarray(np.asarray(x, dtype=np.float32))
    W_qkv = np.ascontiguousarray(np.asarray(W_qkv, dtype=np.float32))
    b_qkv = np.ascontiguousarray(np.asarray(b_qkv, dtype=np.float32))
    W_out = np.ascontiguousarray(np.asarray(W_out, dtype=np.float32))
    b_out = np.ascontiguousarray(np.asarray(b_out, dtype=np.float32))
    table = np.ascontiguousarray(np.asarray(rel_bias_table, dtype=np.float32))
    ids = np.asarray(rel_pos_ids, dtype=np.int32).reshape(-1)

    idsw = _make_idsw(ids)

    if "nc" not in _CACHE:
        _CACHE["nc"] = _build(BPC)
    nc = _CACHE["nc"]

    in_maps = []
    for c in range(N_CORES):
        xs = x[c * BPC:(c + 1) * BPC]                      # [BPC, 256, 768]
        xT = np.ascontiguousarray(xs.transpose(0, 2, 1))   # [BPC, 768, 256]
        in_maps.append({
            "xT": xT,
            "wqkv": W_qkv,
            "bqkv": b_qkv,
            "wout": W_out,
            "bout": b_out,
            "table": table,
            "idsw": idsw,
        })

    _CACHE["in_maps"] = in_maps
    res = run_bass_kernel_spmd(nc, in_maps, core_ids=list(range(N_CORES)))
    out = np.stack([r["out"] for r in res.results], axis=0)  # [8, BPC*256, 768]
    return out.reshape(B, N, E).astype(np.float32)


if __name__ == "__main__":
    rng = np.random.default_rng(0)
    ids = rng.integers(0, 961, 65536).astype(np.int32)
    table = rng.standard_normal((961, H)).astype(np.float32)
    idsw = _make_idsw(ids)
    # emulate indirect_copy + DRAM bounce + strided reload
    bias_dram = np.zeros((128, 8192), np.float32)
    for c in range(8):
        for k in range(8):
            blk = idsw[c * 16:(c + 1) * 16, k * 64:(k + 1) * 64]
            unwrapped = blk.T.reshape(-1)  # [1024]
            for e in range(H):
                bias_dram[c * 16 + e, k * 1024:(k + 1) * 1024] = \
                    table[unwrapped, e]
    flat = bias_dram.reshape(-1)
    bd5 = flat.reshape(2, 4, 16, 32, 256).transpose(0, 1, 3, 2, 4)
    bias_sb = np.zeros((128, H, 512), np.float32)
    for r in range(4):
        for t in range(2):
            bias_sb[r * 32:(r + 1) * 32, :, t * 256:(t + 1) * 256] = \
                bd5[t, r, :, 0:H, :]
    # check vs reference layout: bias_sb[p, h, t*256+m] == bias[h, t*128+p, m]
    ref = table[ids].reshape(256, 256, H).transpose(2, 0, 1)
    err = 0.0
    for t in range(2):
        for p in range(128):
            err = max(err, np.abs(
                bias_sb[p, :, t * 256:(t + 1) * 256] - ref[:, t * 128 + p, :]
            ).max())
    print("bias gather layout check:", err)
